# revision 28
# baseline (speedup 1.0000x reference)
"""Trainium2 Bass kernel for the Haar-mask MLP (histogram_binning).

Every Haar interval edge is a multiple of 2^-10, so the reference's masks --
and therefore the entire MLP output -- depend only on u = floor(t * 1024)
(exact in fp32).  The network collapses to a 1024x3 lookup table computed on
host from the tiny weights; the device work is: stream t, compute u, gather
LUT[u], stream out.

Two gather paths run CONCURRENTLY per core:
  - SWDGE dma_gather (queues 0-3): Q7 core-pairs generate SDMA descriptors,
    SDMA engines pull 12-byte LUT rows from HBM (~2.6 ns/elem aggregate).
    Ring is 128 descriptors, so 512-idx instructions; the engine stalls
    inside the ucode when a queue's ring is still draining...
  - ...so resident gpsimd indirect_copy chunks (~3.5 ns/elem across the 8 Q7
    cores, planar SBUF table) are interleaved between gather rounds to do
    useful work during exactly those drain windows.

Layout per core (16384 elements, wrapped idx layout: idx j of a region sits
at partition 16g + j%16, column j//16):
  - columns [0, DCOL): dma region, REPLICATED across the 8 groups g (each
    queue's tx core reads its own group's copy).  Element = 16*col + b.
  - columns [DCOL, TCOL): ic region, groups hold DISTINCT elements.
  - dma dst [128, DN/128, 3]: dma element jj of instr k -> [jj%128, 4k+jj//128].
  - ic out [128, 16*ICOL]: feature f of (group c, col s, lane b) at
    [16c+f, (s-s0)*16+b] per chunk.
"""

import numpy as np
from contextlib import ExitStack

from concourse import bacc, mybir
from concourse.bass_utils import run_bass_kernel_spmd
from concourse.library_config import mlp as mlp_lib

N_CORES = 8
B, T, F = 16, 8192, 3
N = B * T                    # 131072 total elements
NPC = N // N_CORES           # 16384 per neuron core
P = 128
NBINS = 1024
ROWW = 64                    # LUT row stride: 64 f32 = 256 B (SDMA stride unit)
NQ = 4                       # SWDGE queues (ucode max)
IPG = 512                    # indices per dma_gather (33 ring descs)

NROUND = 5                   # dma rounds (NQ gathers each) / ic chunks
DN = NROUND * NQ * IPG       # dma-path elements: 10240
IN_ = NPC - DN               # ic-path elements: 6144
DCOL = DN // 16              # 640 dma idx columns
ICOL = IN_ // P              # 48 ic columns
TCOL = DCOL + ICOL           # 688 total columns
IC_CH = [10, 10, 10, 10, 8]  # ic cols per chunk (even: 4B-aligned idx)
assert sum(IC_CH) == ICOL and len(IC_CH) == NROUND

IMPL = "hy"
RUN_KWARGS = {}
LAST_RESULTS = None
_CACHE = {}


def _build_lut(W1, b1, W2, b2, W3, b3):
    """MLP output for each of the 1024 half-interval bins, fp32 math."""
    u = np.arange(NBINS)
    acc = np.zeros((NBINS, W1.shape[1]), np.float32)
    for j in range(10):
        k = u >> (10 - j)                       # floor(t * 2^j) for t in bin u
        idx = (1 << j) - 1 + k                  # level-j block offset + k
        sign = np.where((u >> (9 - j)) & 1 == 0, np.float32(1), np.float32(-1))
        acc = acc + sign[:, None] * W1[idx]
    h = np.maximum(acc + b1, np.float32(0))
    h = np.maximum(h @ W2 + b2, np.float32(0))
    return (h @ W3 + b3).astype(np.float32)     # (1024, 3)


def _dma_gather_raw(gp, out_ap, in_ap, idxs_ap, num_idxs, elem_size, elem_step,
                    queue_num):
    """gpsimd.dma_gather minus the elem_size_bytes%256 assert (non-transpose
    HBM path: only the row STRIDE must be a 256B multiple, not the payload)."""
    _in_ap = gp.lower_ap_dma(in_ap, for_custom_bir_dma=True)
    return gp.add_instruction(mybir.InstDMAGatherAnt(
        name=gp.bass.get_next_instruction_name(),
        ins=[*_in_ap, gp.lower_ap(idxs_ap),
             gp.lower_val_access(gp.to_reg(num_idxs))],
        outs=[gp.lower_ap(out_ap)],
        transpose=False, num_idxs=num_idxs, elem_size=elem_size,
        stride_bytes_256=elem_step * 4 // 256, gen_mode=0, single_packet=True,
        queue_num=queue_num, sbuf_tokens_per_rank=0, sbuf_free_dim_per_rank=0,
        sbuf_free_dim_pad_per_rank=0, sbuf_byte_offset=0))


def _build_nc():
    nc = bacc.Bacc("TRN2", target_bir_lowering=False, debug=False,
                   enable_asserts=False, num_devices=N_CORES,
                   num_swdge_queues=NQ)
    f32 = mybir.dt.float32
    u16 = mybir.dt.uint16
    t_d = nc.dram_tensor("t", [P, TCOL], f32, kind="ExternalInput")
    lut_d = nc.dram_tensor("lut", [NBINS, ROWW], f32, kind="ExternalInput")
    plut_d = nc.dram_tensor("plut", [P, NBINS], f32, kind="ExternalInput")
    outd_d = nc.dram_tensor("outd", [P, DN // P, 4], f32, kind="ExternalOutput")
    outi_d = nc.dram_tensor("outi", [P, 16 * ICOL], f32, kind="ExternalOutput")

    spg = IPG // P                               # dst slots per gather: 4
    cpg = IPG // 16                              # idx cols per gather: 32

    with nc.Block() as block, ExitStack() as ctx:
        sb = lambda name, shape, dt: ctx.enter_context(
            nc.sbuf_tensor(name, shape, dt))
        sem = lambda name: ctx.enter_context(nc.semaphore(name))
        t_sb = sb("t_sb", [P, TCOL], f32)
        plut = sb("plut_sb", [P, NBINS], f32)
        uf = sb("uf", [P, TCOL], f32)
        ii = sb("ii", [P, TCOL], mybir.dt.int32)
        fb = sb("fb", [P, TCOL], f32)
        adj = sb("adj", [P, TCOL], f32)
        idx = sb("idx", [P, TCOL], mybir.dt.uint16)
        dst = sb("dst", [P, DN // P, 4], f32)
        icos = [sb(f"ico{r}", [P, 16 * IC_CH[r]], f32) for r in range(NROUND)]
        io, pl, vs, ics, ou = [sem(n) for n in ("io", "pl", "vs", "ics", "ou")]
        qsems = [sem(f"q{q}") for q in range(NQ)]

        @block.sync
        def _(s):
            s.dma_start(t_sb[0:64, :], t_d[0:64, :]).then_inc(io, 16)
            s.dma_start(plut[0:64, :], plut_d[0:64, :]).then_inc(pl, 16)

        @block.scalar
        def _(s):
            s.dma_start(t_sb[64:128, :], t_d[64:128, :]).then_inc(io, 16)
            s.dma_start(plut[64:128, :], plut_d[64:128, :]).then_inc(pl, 16)

        # index chain in chunks: dma cols first (gates gathers), ic cols last
        chunks = [(0, 160), (160, 320), (320, 480), (480, TCOL)]

        @block.vector
        def _(v):
            v.wait_ge(io, 32)
            for c0, c1 in chunks:
                sl = slice(c0, c1)
                # exact floor(t*1024): round-to-int (any rounding mode), then
                # subtract 1 wherever the rounded value exceeds the true value
                v.tensor_scalar(uf[:, sl], t_sb[:, sl], 1024.0, None,
                                mybir.AluOpType.mult)
                v.tensor_copy(ii[:, sl], uf[:, sl])
                v.tensor_copy(fb[:, sl], ii[:, sl])
                v.tensor_tensor(adj[:, sl], fb[:, sl], uf[:, sl],
                                mybir.AluOpType.is_gt)
                v.tensor_sub(fb[:, sl], fb[:, sl], adj[:, sl])
                v.tensor_scalar(idx[:, sl], fb[:, sl], 1023.0, 0.0,
                                mybir.AluOpType.min,
                                mybir.AluOpType.max).then_inc(vs, 1)

        @block.gpsimd
        def _(gp):
            gp.load_library(mlp_lib)
            for k in range(NROUND * NQ):
                need = ((k + 1) * cpg + 159) // 160  # chain chunks needed
                gp.wait_ge(vs, min(need, 4))
                _dma_gather_raw(
                    gp, dst[:, k * spg:(k + 1) * spg, :], lut_d[:, 0:4],
                    idx[:, k * cpg:(k + 1) * cpg], IPG, 4, ROWW,
                    k % NQ).then_inc(qsems[k % NQ], 16)
            gp.wait_ge(vs, 4)
            gp.wait_ge(pl, 32)
            icc = 0
            for r in range(NROUND):
                w = IC_CH[r]
                gp.indirect_copy(
                    icos[r][:].rearrange("p (n d) -> p n d", d=1),
                    plut[:].rearrange("p (n d) -> p n d", d=1),
                    idx[:, DCOL + icc:DCOL + icc + w],
                    i_know_ap_gather_is_preferred=True).then_inc(ics, 1)
                icc += w

        @block.sync
        def _(s):
            for r in range(NROUND):
                for q in range(NQ):
                    s.wait_ge(qsems[q], 16 * (r + 1))
                sl = slice(r * spg * NQ, (r + 1) * spg * NQ)
                s.dma_start(outd_d.ap()[:, sl, :], dst[:, sl, :]).then_inc(ou, 16)
            s.wait_ge(ics, NROUND)
            icc2 = 0
            for r in range(NROUND):
                w = IC_CH[r]
                s.dma_start(outi_d[:, 16 * icc2:16 * (icc2 + w)],
                            icos[r][:]).then_inc(ou, 16)
                icc2 += w
            s.wait_ge(ou, 16 * (2 * NROUND))
    nc.compile()
    return nc


def _host_inputs(t, lut):
    tf = np.ascontiguousarray(np.asarray(t, np.float32)).reshape(N_CORES, NPC)
    tperm = np.zeros((N_CORES, P, TCOL), np.float32)
    # dma region: element j in [0, DN): all groups hold t[j] at
    # [16g + j%16, j//16]
    dmap = tf[:, :DN].reshape(N_CORES, DCOL, 16).transpose(0, 2, 1)
    for g in range(8):
        tperm[:, 16 * g:16 * g + 16, :DCOL] = dmap
    # ic region: element DN + (s*128 + 16g + b) at [16g + b, DCOL + s]
    icm = (tf[:, DN:].reshape(N_CORES, ICOL, P).transpose(0, 2, 1))
    tperm[:, :, DCOL:] = icm
    lutp = np.zeros((NBINS, ROWW), np.float32)
    lutp[:, :F] = lut
    plut = np.ascontiguousarray(lut.T[np.arange(P) % 16 % 3])
    return tperm, lutp, plut


def kernel(t, W1, b1, W2, b2, W3, b3):
    global LAST_RESULTS
    key = ("nc", IMPL)
    if key not in _CACHE:
        _CACHE[key] = _build_nc()
    nc = _CACHE[key]

    lut = _build_lut(np.asarray(W1, np.float32), np.asarray(b1, np.float32),
                     np.asarray(W2, np.float32), np.asarray(b2, np.float32),
                     np.asarray(W3, np.float32), np.asarray(b3, np.float32))
    tperm, lutp, plut = _host_inputs(t, lut)
    in_maps = [{"t": np.ascontiguousarray(tperm[m]), "lut": lutp, "plut": plut}
               for m in range(N_CORES)]

    res = run_bass_kernel_spmd(nc, in_maps, list(range(N_CORES)), **RUN_KWARGS)
    LAST_RESULTS = res

    out = np.empty((N_CORES, NPC, F), np.float32)
    for m in range(N_CORES):
        od = res.results[m]["outd"][:, :, :F]    # [128, DN//128, 4->3]
        # dma element j -> [j%128, j//128]
        out[m, :DN] = od.transpose(1, 0, 2).reshape(DN, F)
        oi = res.results[m]["outi"].reshape(P, ICOL, 16)  # [p, s, b']
        # ic element DN + s*128 + 16c + b: feature f at [16c+f, s*16+b]
        oi4 = oi.reshape(8, 16, ICOL, 16)        # [c, f-lane, s, b]
        ic = oi4[:, :F].transpose(2, 0, 3, 1)    # [s, c, b, f]
        out[m, DN:] = ic.reshape(IN_, F)
    return out.reshape(B, T, F).astype(np.float32)
